# revision 13
# baseline (speedup 1.0000x reference)
"""CrossAttention kernel for 8 Trainium2 NeuronCores.

Problem (hardcoded): B=4, F=T=1024, C=1024, H=16, D=64, fp32 I/O.
    qh = (q @ Wq.T + bq) split into heads          [B,H,F,D]
    kh = (k @ Wk.T + bk)                           [B,H,T,D]
    vh = (v @ Wv.T + bv)                           [B,H,T,D]
    s  = qh @ kh.T / 8 + rel_bias;  s[mask] = -1e4
    out = softmax(s) @ vh  -> merge heads -> @ Wo.T + bo

Sharding: core c handles batch b=c//2 and head-half hg=c%2 (8 heads).
Each core computes a partial output (its 8 heads' contribution to
out[b] including the bv fold-in); host sums the two partials per batch
and adds bo.

Device-side layout choice (prepared on host as part of sharding):
inputs are passed pre-transposed and cast to bf16 so that the
contraction dim always lands on SBUF partitions:
    qT/kT/vT   [C, F|T]   bf16
    wqT/wkT/wvT [C, 512]  bf16 (head-half column slice of W.T)
    woT        [512, C]   bf16
    biasT8     [8, T, F]  bf16  (= 8 * rel_bias, transposed per head)
    maskTn     [T, F]     bf16  (= -81920 * mask)
Scores are computed transposed (sT[t,f]) so softmax's reduction over t
becomes a matmul; bias/mask are accumulated into the score PSUM with
identity matmuls; exp applies the 1/8 scale; the softmax denominator
comes from a ones-column appended to vh; normalization happens on the
small [D,F] attention output, not the [T,F] probability matrix.
"""

import math
import numpy as np
import ml_dtypes

B, F, T, C, H, D = 4, 1024, 1024, 1024, 16, 64
NCORES = 8
HPC = H // 2          # heads per core = 8
CPC = HPC * D         # channel cols per core = 512
NPAIR = HPC // 2      # head pairs per core = 4
FT = F // 512         # f tiles of 512
TB = T // 128         # t blocks of 128
CB = C // 128         # c blocks of 128

bf16 = ml_dtypes.bfloat16

_COMPILED = {}


def build_program(taps=False):
    import concourse.bass as bass
    import concourse.mybir as mybir
    import concourse.tile as tile
    from concourse import bacc
    from concourse.masks import make_identity

    dt = mybir.dt
    AF = mybir.ActivationFunctionType
    ALU = mybir.AluOpType

    nc = bacc.Bacc(
        "TRN2", target_bir_lowering=False, debug=False, num_devices=NCORES
    )

    def din(name, shape, dtype=dt.bfloat16):
        return nc.dram_tensor(name, shape, dtype, kind="ExternalInput").ap()

    qT = din("qT", [C, F])
    kT = din("kT", [C, T])
    vT = din("vT", [C, T])
    wqT = din("wqT", [C, CPC])
    wkT = din("wkT", [C, CPC])
    wvT = din("wvT", [C, CPC])
    woT = din("woT", [CPC, C])
    bq2 = din("bq2", [128, NPAIR], dt.float32)
    bk2 = din("bk2", [128, NPAIR], dt.float32)
    bvrow = din("bvrow", [1, C])
    biasT8 = din("biasT8", [HPC, T, F])
    maskTn = din("maskTn", [T, F])
    out_d = nc.dram_tensor("out_part", [F, C], dt.float32, kind="ExternalOutput").ap()
    # internal DRAM scratch for the reciprocal partition-broadcast bounce
    rc_scr = nc.dram_tensor("rc_scr", [NPAIR * FT * 2, 512], dt.float32).ap()
    tap = {}
    if taps:
        for nm, shape in [
            ("tap_qhT", [128, NPAIR, F]),
            ("tap_khT", [128, NPAIR, T]),
            ("tap_vhE", [128, TB, NPAIR, 65]),
            ("tap_pt1", [128, TB, 512]),
            ("tap_aoT", [128, NPAIR, F]),
        ]:
            tap[nm] = nc.dram_tensor(
                nm, shape, dt.bfloat16, kind="ExternalOutput"
            ).ap()
        tap["tap_rc"] = nc.dram_tensor(
            "tap_rc", [128, 512], dt.float32, kind="ExternalOutput"
        ).ap()

    with tile.TileContext(nc) as tc:
        with (
            tc.tile_pool(name="const", bufs=1) as constp,
            tc.tile_pool(name="inp", bufs=1) as inp,
            tc.tile_pool(name="bias", bufs=2) as biasp,
            tc.tile_pool(name="proj", bufs=1) as projp,
            tc.tile_pool(name="pt", bufs=2) as ptp,
            tc.tile_pool(name="rc", bufs=4) as rcp,
            tc.tile_pool(name="fin", bufs=2) as finp,
            tc.tile_pool(name="psA", bufs=2, space="PSUM") as psA,
            tc.tile_pool(name="psS", bufs=4, space="PSUM") as psS,
            tc.tile_pool(name="psO", bufs=2, space="PSUM") as psO,
        ):
            # ---- constants ----
            ident = constp.tile([128, 128], dt.bfloat16, name="ident")
            make_identity(nc, ident[:, :])
            ones_row = constp.tile([1, 128], dt.bfloat16, name="ones_row")
            nc.gpsimd.memset(ones_row[:, :], 1.0)
            bq_sb = constp.tile([128, NPAIR], dt.float32, name="bq_sb")
            bk_sb = constp.tile([128, NPAIR], dt.float32, name="bk_sb")
            bv_sb = constp.tile([1, C], dt.bfloat16, name="bv_sb")
            nc.sync.dma_start(bq_sb[:, :], bq2)
            nc.sync.dma_start(bk_sb[:, :], bk2)
            nc.sync.dma_start(bv_sb[:, :], bvrow)

            # ---- resident inputs ----
            qT_sb = inp.tile([128, CB, F], dt.bfloat16, name="qT_sb")
            kT_sb = inp.tile([128, CB, T], dt.bfloat16, name="kT_sb")
            vT_sb = inp.tile([128, CB, T], dt.bfloat16, name="vT_sb")
            wq_sb = inp.tile([128, CB, CPC], dt.bfloat16, name="wq_sb")
            wk_sb = inp.tile([128, CB, CPC], dt.bfloat16, name="wk_sb")
            wv_sb = inp.tile([128, CB, CPC], dt.bfloat16, name="wv_sb")
            wo_sb = inp.tile([128, NPAIR, C], dt.bfloat16, name="wo_sb")
            mask_sb = inp.tile([128, TB, F], dt.bfloat16, name="mask_sb")
            nc.sync.dma_start(qT_sb[:, :, :], qT.rearrange("(cb p) f -> p cb f", p=128))
            nc.sync.dma_start(kT_sb[:, :, :], kT.rearrange("(cb p) f -> p cb f", p=128))
            nc.sync.dma_start(vT_sb[:, :, :], vT.rearrange("(cb p) f -> p cb f", p=128))
            nc.sync.dma_start(wq_sb[:, :, :], wqT.rearrange("(cb p) d -> p cb d", p=128))
            nc.sync.dma_start(wk_sb[:, :, :], wkT.rearrange("(cb p) d -> p cb d", p=128))
            nc.sync.dma_start(wv_sb[:, :, :], wvT.rearrange("(cb p) d -> p cb d", p=128))
            nc.sync.dma_start(wo_sb[:, :, :], woT.rearrange("(jb p) o -> p jb o", p=128))
            nc.sync.dma_start(
                mask_sb[:, :, :], maskTn.rearrange("(tb p) f -> p tb f", p=128)
            )

            # ---- projections ----
            # qhT/khT: [128 (pair of head dims), pair, F] bf16
            qhT = projp.tile([128, NPAIR, F], dt.bfloat16, name="qhT")
            khT = projp.tile([128, NPAIR, T], dt.bfloat16, name="khT")
            for pair in range(NPAIR):
                for ft in range(FT):
                    fs = slice(512 * ft, 512 * (ft + 1))
                    pq = psA.tile([128, 512], dt.float32, tag="psA")
                    for cb in range(CB):
                        nc.tensor.matmul(
                            pq[:, :],
                            lhsT=wq_sb[:, cb, 128 * pair : 128 * (pair + 1)],
                            rhs=qT_sb[:, cb, fs],
                            start=(cb == 0),
                            stop=(cb == CB - 1),
                        )
                    nc.scalar.activation(
                        qhT[:, pair, fs], pq[:, :], AF.Identity,
                        bias=bq_sb[:, pair : pair + 1],
                    )
                    pk = psA.tile([128, 512], dt.float32, tag="psA")
                    for cb in range(CB):
                        nc.tensor.matmul(
                            pk[:, :],
                            lhsT=wk_sb[:, cb, 128 * pair : 128 * (pair + 1)],
                            rhs=kT_sb[:, cb, fs],
                            start=(cb == 0),
                            stop=(cb == CB - 1),
                        )
                    nc.scalar.activation(
                        khT[:, pair, fs], pk[:, :], AF.Identity,
                        bias=bk_sb[:, pair : pair + 1],
                    )

            # vh: even heads augmented with a ones column [vh | 1] (their
            # attn@v matmul sits at base 0 with M=65, row 64 = denominator);
            # odd heads kept plain (M=64 at base 64; denominator comes from a
            # separate ones-lhsT M=1 matmul into row 0).
            vhE = projp.tile([128, TB, NPAIR, 65], dt.bfloat16, name="vhE")
            vhN = projp.tile([128, TB, NPAIR, 64], dt.bfloat16, name="vhN")
            nc.gpsimd.memset(vhE[:, :, :, 64:65], 1.0)
            ones_col = constp.tile([128, 1], dt.bfloat16, name="ones_col")
            nc.gpsimd.memset(ones_col[:, :], 1.0)
            for tb in range(TB):
                pv = psA.tile([128, 512], dt.float32, tag="psA")
                for cb in range(CB):
                    nc.tensor.matmul(
                        pv[:, :],
                        lhsT=vT_sb[:, cb, 128 * tb : 128 * (tb + 1)],
                        rhs=wv_sb[:, cb, :],
                        start=(cb == 0),
                        stop=(cb == CB - 1),
                    )
                # even heads 0,2,4,6 -> cols [h*64, h*64+64)
                nc.vector.tensor_copy(
                    vhE[:, tb, :, 0:64],
                    pv[:, :].rearrange("p (hp two d) -> p hp two d", two=2, d=64)[
                        :, :, 0, :
                    ],
                )
                nc.vector.tensor_copy(
                    vhN[:, tb, :, :],
                    pv[:, :].rearrange("p (hp two d) -> p hp two d", two=2, d=64)[
                        :, :, 1, :
                    ],
                )

            if taps:
                nc.sync.dma_start(tap["tap_qhT"], qhT[:, :, :])
                nc.sync.dma_start(tap["tap_khT"], khT[:, :, :])
                nc.sync.dma_start(tap["tap_vhE"], vhE[:, :, :, :])

            # ---- attention ----
            aoT = projp.tile([128, NPAIR, F], dt.bfloat16, name="aoT")
            for pair in range(NPAIR):
                h1, h2 = 2 * pair, 2 * pair + 1
                bias1 = biasp.tile([128, TB, F], dt.bfloat16, tag="biasT")
                bias2 = biasp.tile([128, TB, F], dt.bfloat16, tag="biasT")
                nc.sync.dma_start(
                    bias1[:, :, :], biasT8[h1].rearrange("(tb p) f -> p tb f", p=128)
                )
                nc.sync.dma_start(
                    bias2[:, :, :], biasT8[h2].rearrange("(tb p) f -> p tb f", p=128)
                )
                for ft in range(FT):
                    fs = slice(512 * ft, 512 * (ft + 1))
                    pt1 = ptp.tile([128, TB, 512], dt.bfloat16, tag="pt")
                    pt2 = ptp.tile([128, TB, 512], dt.bfloat16, tag="pt")
                    for tb in range(TB):
                        ts_ = slice(128 * tb, 128 * (tb + 1))
                        ps1 = psS.tile([128, 512], dt.float32, tag="psS")
                        ps2 = psS.tile([128, 512], dt.float32, tag="psS")
                        nc.tensor.matmul(
                            ps1[:, :], lhsT=khT[0:64, pair, ts_],
                            rhs=qhT[0:64, pair, fs], start=True, stop=False,
                        )
                        nc.tensor.matmul(
                            ps2[:, :], lhsT=khT[64:128, pair, ts_],
                            rhs=qhT[64:128, pair, fs], start=True, stop=False,
                        )
                        nc.tensor.matmul(
                            ps1[:, :], lhsT=ident[:, :], rhs=bias1[:, tb, fs],
                            start=False, stop=False,
                        )
                        nc.tensor.matmul(
                            ps1[:, :], lhsT=ident[:, :], rhs=mask_sb[:, tb, fs],
                            start=False, stop=True,
                        )
                        nc.tensor.matmul(
                            ps2[:, :], lhsT=ident[:, :], rhs=bias2[:, tb, fs],
                            start=False, stop=False,
                        )
                        nc.tensor.matmul(
                            ps2[:, :], lhsT=ident[:, :], rhs=mask_sb[:, tb, fs],
                            start=False, stop=True,
                        )
                        nc.scalar.activation(
                            pt1[:, tb, :], ps1[:, :], AF.Exp, scale=0.125
                        )
                        nc.scalar.activation(
                            pt2[:, tb, :], ps2[:, :], AF.Exp, scale=0.125
                        )
                    # attn @ v for the two heads of this pair
                    po1 = psO.tile([128, 512], dt.float32, tag="psO")
                    po2 = psO.tile([128, 512], dt.float32, tag="psO")
                    for tb in range(TB):
                        nc.tensor.matmul(
                            po1[0:65, :], lhsT=vhE[:, tb, pair, :],
                            rhs=pt1[:, tb, :], start=(tb == 0), stop=(tb == TB - 1),
                        )
                        nc.tensor.matmul(
                            po2[64:128, :], lhsT=vhN[:, tb, pair, :],
                            rhs=pt2[:, tb, :], start=(tb == 0), stop=(tb == TB - 1),
                        )
                        nc.tensor.matmul(
                            po2[0:1, :], lhsT=ones_col[:, :],
                            rhs=pt2[:, tb, :], start=(tb == 0), stop=(tb == TB - 1),
                            skip_group_check=True,
                        )
                    if taps and pair == 0 and ft == 0:
                        nc.sync.dma_start(tap["tap_pt1"], pt1[:, :, :])
                    rc1 = rcp.tile([128, 512], dt.float32, tag="rc")
                    rc2 = rcp.tile([128, 512], dt.float32, tag="rc")
                    row1 = (pair * FT + ft) * 2
                    bc1 = bass.AP(
                        tensor=rc_scr.tensor, offset=row1 * 512,
                        ap=[[0, 64], [1, 512]],
                    )
                    bc2 = bass.AP(
                        tensor=rc_scr.tensor, offset=(row1 + 1) * 512,
                        ap=[[0, 64], [1, 512]],
                    )
                    nc.vector.reciprocal(rc1[64:65, :], po1[64:65, :])
                    nc.sync.dma_start(rc_scr[row1 : row1 + 1, :], rc1[64:65, :])
                    nc.gpsimd.dma_start(rc1[0:64, :], bc1)
                    nc.vector.tensor_mul(
                        aoT[0:64, pair, fs], po1[0:64, :], rc1[0:64, :]
                    )
                    nc.vector.reciprocal(rc2[0:1, :], po2[0:1, :])
                    nc.sync.dma_start(rc_scr[row1 + 1 : row1 + 2, :], rc2[0:1, :])
                    nc.gpsimd.dma_start(rc2[64:128, :], bc2)
                    nc.vector.tensor_mul(
                        aoT[64:128, pair, fs], po2[64:128, :], rc2[64:128, :]
                    )
                    if taps and pair == 0 and ft == 0:
                        nc.sync.dma_start(tap["tap_rc"][0:64, :], rc1[0:64, :])

            if taps:
                nc.sync.dma_start(tap["tap_aoT"], aoT[:, :, :])

            # ---- output projection (partial: this core's 8 heads + bv fold) ----
            for fb in range(F // 128):
                fin = finp.tile([128, 1024], dt.float32, tag="fin")
                for ot in range(2):
                    os_ = slice(512 * ot, 512 * (ot + 1))
                    pf = psA.tile([128, 512], dt.float32, tag="psA")
                    for pair in range(NPAIR):
                        nc.tensor.matmul(
                            pf[:, :],
                            lhsT=aoT[:, pair, 128 * fb : 128 * (fb + 1)],
                            rhs=wo_sb[:, pair, os_],
                            start=(pair == 0),
                            stop=False,
                        )
                    nc.tensor.matmul(
                        pf[:, :], lhsT=ones_row[:, :], rhs=bv_sb[:, os_],
                        start=False, stop=True,
                    )
                    nc.vector.tensor_copy(fin[:, os_], pf[:, :])
                nc.sync.dma_start(out_d[128 * fb : 128 * (fb + 1), :], fin[:, :])

    nc.compile()
    return nc


def get_program():
    if "nc" not in _COMPILED:
        _COMPILED["nc"] = build_program()
    return _COMPILED["nc"]


def shard_inputs(q, k, v, attn_mask, rel_bias, Wq, bq, Wk, bk, Wv, bv, Wo, bo):
    """Build the 8 per-core input maps (host-side sharding + layout)."""
    in_maps = []
    for core in range(NCORES):
        b, hg = core // 2, core % 2
        sl = slice(hg * CPC, (hg + 1) * CPC)
        heads = slice(hg * HPC, (hg + 1) * HPC)
        m = {
            "qT": np.ascontiguousarray(q[b].T).astype(bf16),
            "kT": np.ascontiguousarray(k[b].T).astype(bf16),
            "vT": np.ascontiguousarray(v[b].T).astype(bf16),
            "wqT": np.ascontiguousarray(Wq[sl].T).astype(bf16),
            "wkT": np.ascontiguousarray(Wk[sl].T).astype(bf16),
            "wvT": np.ascontiguousarray(Wv[sl].T).astype(bf16),
            "woT": np.ascontiguousarray(Wo[:, sl].T).astype(bf16),
            "bq2": np.ascontiguousarray(
                bq[sl].reshape(NPAIR, 128).T
            ).astype(np.float32),
            "bk2": np.ascontiguousarray(
                bk[sl].reshape(NPAIR, 128).T
            ).astype(np.float32),
            "bvrow": (Wo[:, sl].astype(np.float64) @ bv[sl].astype(np.float64))
            .astype(np.float32)
            .reshape(1, C)
            .astype(bf16),
            "biasT8": np.ascontiguousarray(
                (8.0 * rel_bias[0, heads]).transpose(0, 2, 1)
            ).astype(bf16),
            "maskTn": np.ascontiguousarray(
                attn_mask[b, 0].T.astype(np.float32) * -81920.0
            ).astype(bf16),
        }
        in_maps.append(m)
    return in_maps


def kernel(q, k, v, attn_mask, rel_bias, Wq, bq, Wk, bk, Wv, bv, Wo, bo):
    from concourse import bass_utils

    args = [np.asarray(x) for x in (q, k, v, attn_mask, rel_bias,
                                    Wq, bq, Wk, bk, Wv, bv, Wo, bo)]
    nc = get_program()
    in_maps = shard_inputs(*args)
    res = bass_utils.run_bass_kernel_spmd(nc, in_maps, list(range(NCORES)))
    parts = [r["out_part"] for r in res.results]
    bo_f = args[12].astype(np.float32)
    out = np.empty((B, F, C), dtype=np.float32)
    for b in range(B):
        out[b] = parts[2 * b] + parts[2 * b + 1] + bo_f
    return out
